# revision 1
# baseline (speedup 1.0000x reference)
"""Trainium2 Bass kernel for nn_Attention (B=2, L=2048, D=1024, H=16 heads).

Sharding (8 cores): data-parallel over batch (2) x tensor-parallel over heads
(4 groups of 4 heads), Megatron-style. Each core computes, for its batch b and
its 4 heads:
    QT/KT = (Wq_s @ x^T)               (transposed-layout projections; 1/8 score
                                        scale pre-folded into Wq/bq on host)
    V     = x_v @ Wv_s^T               (natural layout; key-padding mask folded
                                        into x_v rows on host)
    S^T   = K Q^T per head             (keys on partitions, queries free)
    P^T   = exp(S^T) * causal_mask     (no row-max: scores are O(4))
    [y^T; sums] = [V | kmask]^T P^T    (AV matmul also folds softmax sums)
    y_norm^T = y^T * approx(1/sums)    (gpsimd partition-broadcast of sums row)
    out_partial = y_norm @ Wp_s^T      (column shard of Wp)
Host sums the 4 partials per batch and adds bp + bv @ Wp^T (exact: softmax rows
sum to 1, so the V-bias contribution is a constant row vector).
"""

import numpy as np
import concourse.bass as bass
import concourse.tile as tile
from concourse import bacc, mybir
from concourse.bass import ds, ts
from concourse.bass_utils import run_bass_kernel_spmd

F32 = mybir.dt.float32
F32R = mybir.dt.float32r

B, L, D, H = 2, 2048, 1024, 16
HD = 64          # head dim
HPC = 4          # heads per core
DS = HPC * HD    # 256: per-core shard of D
P = 128
NCORES = 8
LT = L // P      # 16 l-tiles
NJ = L // 512    # 4 q-chunks
NDT = D // P     # 8 contraction tiles over D

_CACHE = {}


def _build():
    nc = bacc.Bacc("TRN2", target_bir_lowering=False, debug=False, num_devices=NCORES)

    xq = nc.declare_dram_parameter("xq", [D, L], F32R, isOutput=False)
    xk = nc.declare_dram_parameter("xk", [D, L], F32R, isOutput=False)
    xv = nc.declare_dram_parameter("xv", [D, L], F32R, isOutput=False)
    wq = nc.declare_dram_parameter("wq", [P, NDT, DS], F32R, isOutput=False)
    wk = nc.declare_dram_parameter("wk", [P, NDT, DS], F32R, isOutput=False)
    wv = nc.declare_dram_parameter("wv", [P, NDT, DS], F32R, isOutput=False)
    wp = nc.declare_dram_parameter("wp", [P, 2, D], F32R, isOutput=False)
    bqp = nc.declare_dram_parameter("bq", [P, 2], F32, isOutput=False)
    bkp = nc.declare_dram_parameter("bk", [P, 2], F32, isOutput=False)
    emask = nc.declare_dram_parameter("emask", [P, LT], F32, isOutput=False)
    cmask = nc.declare_dram_parameter("cmask", [P, 4, 512], F32R, isOutput=False)
    out = nc.declare_dram_parameter("out", [L, D], F32, isOutput=True)

    with tile.TileContext(nc) as tc:
        with tc.tile_pool(name="consts", bufs=1) as consts, \
             tc.tile_pool(name="state", bufs=1) as state, \
             tc.tile_pool(name="xp", bufs=10) as xp, \
             tc.tile_pool(name="ptp", bufs=8) as ptp, \
             tc.tile_pool(name="rp", bufs=2) as rp, \
             tc.tile_pool(name="op", bufs=2) as op, \
             tc.tile_pool(name="pw", bufs=3, space="PSUM") as pw, \
             tc.tile_pool(name="py", bufs=2, space="PSUM") as py:

            # ---- constants ----
            wq_sb = consts.tile([P, NDT, DS], F32R)
            wk_sb = consts.tile([P, NDT, DS], F32R)
            wv_sb = consts.tile([P, NDT, DS], F32R)
            for (wsb_, wdr_) in ((wv_sb, wv), (wq_sb, wq), (wk_sb, wk)):
                for dt2 in range(0, NDT, 2):
                    nc.scalar.dma_start(wsb_[:, dt2:dt2 + 2, :], wdr_[:, dt2:dt2 + 2, :])
            wp_sb = consts.tile([P, 2, D], F32R)
            nc.scalar.dma_start(wp_sb[:], wp[:])
            bq_sb = consts.tile([P, 2], F32)
            bk_sb = consts.tile([P, 2], F32)
            nc.scalar.dma_start(bq_sb[:], bqp[:])
            nc.scalar.dma_start(bk_sb[:], bkp[:])
            em_sb = consts.tile([P, LT], F32)
            nc.scalar.dma_start(em_sb[:], emask[:])
            cm_sb = consts.tile([P, 4, 512], F32R)
            nc.scalar.dma_start(cm_sb[:], cmask[:])

            # ---- PE warm-up during input DMA lead-in (results never read) ----
            wu = consts.tile([P, 512], mybir.dt.bfloat16)
            nc.any.memset(wu[:], 0.25)
            for i in range(32):
                pwu = pw.tile([P, 512], F32, tag="pw")
                nc.tensor.matmul(pwu[:], wu[:, 0:128], wu[:], start=True, stop=True)

            # ---- big state ----
            QT = state.tile([P, 2, L], F32R)       # [dout pair, chunk, l]
            KT = state.tile([P, 2, L], F32R)
            V = state.tile([P, LT, HPC * (HD + 1)], F32R)  # per head: [V(64) | emask]
            YT = state.tile([P, 2, L], F32R)       # normalized y^T, head pairs

            # "ones" (key-valid mask) columns of V
            for h in range(HPC):
                col = 65 * h + HD
                nc.scalar.copy(V[:, :, col:col + 1], em_sb[:, :, None])

            # ---- V projection: V[l, dout] natural layout ----
            for j in range(NJ):
                xt = []
                for dt in range(NDT):
                    t = xp.tile([P, 512], F32R, tag="x")
                    nc.sync.dma_start(t[:], xv[ds(P * dt, P), ds(512 * j, 512)])
                    xt.append(t)
                for sub in range(4):
                    lt = 4 * j + sub
                    pv = pw.tile([P, DS], F32, tag="pw")
                    for dt in range(NDT):
                        nc.tensor.matmul(pv[:], xt[dt][:, ds(128 * sub, 128)],
                                         wv_sb[:, dt, :], start=(dt == 0),
                                         stop=(dt == NDT - 1))
                    for h in range(HPC):
                        if h % 2 == 0:
                            nc.scalar.copy(V[:, lt, ds(65 * h, HD)],
                                           pv[:, ds(HD * h, HD)])
                        else:
                            nc.vector.tensor_copy(V[:, lt, ds(65 * h, HD)],
                                                  pv[:, ds(HD * h, HD)])

            # ---- Q/K projections, interleaved per j-chunk ----
            for j in range(NJ):
                for (xin, wsb, bsb, dst) in ((xq, wq_sb, bq_sb, QT),
                                             (xk, wk_sb, bk_sb, KT)):
                    xt = []
                    for dt in range(NDT):
                        t = xp.tile([P, 512], F32R, tag="x")
                        nc.sync.dma_start(t[:], xin[ds(P * dt, P), ds(512 * j, 512)])
                        xt.append(t)
                    for dc in range(2):
                        pacc = pw.tile([P, 512], F32, tag="pw")
                        for dt in range(NDT):
                            nc.tensor.matmul(pacc[:], wsb[:, dt, ds(128 * dc, 128)],
                                             xt[dt][:], start=(dt == 0),
                                             stop=(dt == NDT - 1))
                        nc.scalar.activation(dst[:, dc, ds(512 * j, 512)], pacc[:],
                                             mybir.ActivationFunctionType.Identity,
                                             bias=bsb[:, dc:dc + 1])

            # ---- attention per head ----
            for h in range(HPC):
                hc, hrow = h // 2, 64 * (h % 2)
                for j in range(NJ):
                    nk = 4 * j + 4
                    pyt = py.tile([65, 512], F32, tag="py")
                    for blk in range(nk // 2):
                        t0 = 2 * blk
                        # last diagonal pair: q-cols [0:256) of this chunk are
                        # fully causal-masked -> skip computing them
                        trim = 256 if t0 - 4 * j == 2 else 0
                        qw = 512 - trim
                        pwt = pw.tile([P, 1024], F32, tag="pw")
                        for u in range(2):
                            nc.tensor.matmul(pwt[:, ds(512 * u, qw)],
                                             KT[ds(hrow, HD), hc, ts(t0 + u, P)],
                                             QT[ds(hrow, HD), hc,
                                                ds(512 * j + trim, qw)],
                                             start=True, stop=True)
                        pt = ptp.tile([P, 1024], F32R, tag="pt")
                        pwv = pwt[:].rearrange("p (a b) -> p a b", a=2)[:, :, 0:qw]
                        ptv = pt[:, ds(0, 2 * qw)].rearrange("p (a b) -> p a b", a=2)
                        nc.scalar.activation(ptv, pwv,
                                             mybir.ActivationFunctionType.Exp)
                        if t0 >= 4 * j:
                            r = t0 - 4 * j
                            nc.vector.tensor_mul(out=ptv, in0=ptv,
                                                 in1=cm_sb[:, r:r + 2, trim:512])
                        for u in range(2):
                            t = t0 + u
                            nc.tensor.matmul(pyt[:, ds(trim, qw)],
                                             V[:, t, ds(65 * h, HD + 1)],
                                             pt[:, ds(u * qw, qw)],
                                             start=(t == 0), stop=(t == nk - 1))
                    # normalization: R = approx(1/sums) broadcast over partitions
                    rrow = rp.tile([P, 512], F32, tag="rr")
                    nc.vector.tensor_copy(rrow[0:1, :], pyt[64:65, :])
                    Rb = rp.tile([P, 512], F32, tag="rb")
                    nc.gpsimd.partition_broadcast(Rb[:], rrow[0:1, :])
                    Rr = rp.tile([P, 512], F32, tag="rc")
                    nc.vector.reciprocal_approx_fast(Rr[:], Rb[:])
                    nc.vector.tensor_mul(out=YT[ds(hrow, HD), hc, ds(512 * j, 512)],
                                         in0=pyt[0:64, :],
                                         in1=Rr[ds(hrow, HD), :])

            # ---- output projection ----
            for lt in range(LT):
                osb = op.tile([P, D], F32, tag="o")
                for dc in range(2):
                    po = pw.tile([P, 512], F32, tag="pw")
                    for hc in range(2):
                        nc.tensor.matmul(po[:], YT[:, hc, ts(lt, P)],
                                         wp_sb[:, hc, ds(512 * dc, 512)],
                                         start=(hc == 0), stop=(hc == 1))
                    if dc == 0:
                        nc.scalar.copy(osb[:, ds(512 * dc, 512)], po[:])
                    else:
                        nc.vector.tensor_copy(osb[:, ds(512 * dc, 512)], po[:])
                nc.sync.dma_start(out[ts(lt, P), :], osb[:])

    nc.compile()
    return nc


def _get_nc():
    if "nc" not in _CACHE:
        _CACHE["nc"] = _build()
    return _CACHE["nc"]


def _wlayout(w):
    # [D, DS] -> [P, NDT, DS] with row (o*P + p) at [p, o, :]
    return np.ascontiguousarray(w.reshape(NDT, P, DS).transpose(1, 0, 2))


def _wlayout2(w):
    # [DS, D] -> [P, 2, D]
    return np.ascontiguousarray(w.reshape(2, P, D).transpose(1, 0, 2))


def _shard_inputs(query, key, value, kmask, Wq, bq, Wk, bk, Wv, Wp):
    kk = np.arange(P)[:, None, None]
    rr = np.arange(4)[None, :, None]
    qq = np.arange(512)[None, None, :]
    cmask = (P * rr + kk <= qq).astype(np.float32)
    scale = 0.125  # 1/sqrt(HD), folded into the Q projection
    in_maps = []
    for c in range(NCORES):
        b, hg = divmod(c, HPC)
        hs = slice(DS * hg, DS * (hg + 1))
        kvalid = kmask[b].astype(np.float32)
        in_maps.append({
            "xq": np.ascontiguousarray(query[b].T),
            "xk": np.ascontiguousarray(key[b].T),
            "xv": np.ascontiguousarray((value[b] * kvalid[:, None]).T),
            "wq": _wlayout(Wq[hs].T * scale),
            "wk": _wlayout(Wk[hs].T),
            "wv": _wlayout(Wv[hs].T),
            "wp": _wlayout2(Wp[:, hs].T),
            "bq": np.ascontiguousarray((bq[hs] * scale).reshape(2, P).T),
            "bk": np.ascontiguousarray(bk[hs].reshape(2, P).T),
            "emask": np.ascontiguousarray(kvalid.reshape(LT, P).T),
            "cmask": cmask,
        })
    return in_maps


def kernel(query, key, value, kmask, Wq, bq, Wk, bk, Wv, bv, Wp, bp):
    query = np.asarray(query, dtype=np.float32)
    key = np.asarray(key, dtype=np.float32)
    value = np.asarray(value, dtype=np.float32)
    kmask = np.asarray(kmask)
    Wq = np.asarray(Wq, dtype=np.float32)
    bq = np.asarray(bq, dtype=np.float32)
    Wk = np.asarray(Wk, dtype=np.float32)
    bk = np.asarray(bk, dtype=np.float32)
    Wv = np.asarray(Wv, dtype=np.float32)
    bv = np.asarray(bv, dtype=np.float32)
    Wp = np.asarray(Wp, dtype=np.float32)
    bp = np.asarray(bp, dtype=np.float32)

    in_maps = _shard_inputs(query, key, value, kmask, Wq, bq, Wk, bk, Wv, Wp)
    nc = _get_nc()
    res = run_bass_kernel_spmd(nc, in_maps, list(range(NCORES))).results

    outp = np.zeros((B, L, D), dtype=np.float32)
    for c in range(NCORES):
        b = c // HPC
        outp[b] += res[c]["out"]
    outp += bp[None, None, :] + (bv @ Wp.T)[None, None, :]
    return outp



# revision 2
# speedup vs baseline: 1.1677x; 1.1677x over previous
"""Trainium2 Bass kernel for nn_Attention (B=2, L=2048, D=1024, H=16 heads).

Sharding (8 cores): data-parallel over batch (2) x tensor-parallel over heads
(4 groups of 4 heads), Megatron-style. Each core computes, for its batch b and
its 4 heads:
    QT/KT = (Wq_s @ x^T)               (transposed-layout projections; 1/8 score
                                        scale pre-folded into Wq/bq on host)
    V     = x_v @ Wv_s^T               (natural layout; key-padding mask folded
                                        into x_v rows on host)
    S^T   = K Q^T per head             (keys on partitions, queries free)
    P^T   = exp(S^T) * causal_mask     (no row-max: scores are O(4))
    [y^T; sums] = [V | kmask]^T P^T    (AV matmul also folds softmax sums)
    y_norm^T = y^T * approx(1/sums)    (gpsimd partition-broadcast of sums row)
    out_partial = y_norm @ Wp_s^T      (column shard of Wp)
Host sums the 4 partials per batch and adds bp + bv @ Wp^T (exact: softmax rows
sum to 1, so the V-bias contribution is a constant row vector).

All matmul operands are bf16 (fp32 PSUM accumulation): 1 cycle/row streaming
vs 2 for fp32r, FastWeightLoad on 128-col weights, and half the HBM traffic.
"""

import numpy as np
import ml_dtypes
import concourse.bass as bass
import concourse.tile as tile
from concourse import bacc, mybir
from concourse.bass import ds, ts
from concourse.bass_utils import run_bass_kernel_spmd

F32 = mybir.dt.float32
BF16 = mybir.dt.bfloat16

B, L, D, H = 2, 2048, 1024, 16
HD = 64          # head dim
HPC = 4          # heads per core
DS = HPC * HD    # 256: per-core shard of D
P = 128
NCORES = 8
LT = L // P      # 16 l-tiles
NJ = L // 512    # 4 q-chunks
NDT = D // P     # 8 contraction tiles over D

_CACHE = {}


def _build():
    nc = bacc.Bacc("TRN2", target_bir_lowering=False, debug=False, num_devices=NCORES)

    xq = nc.declare_dram_parameter("xq", [D, L], BF16, isOutput=False)
    xk = nc.declare_dram_parameter("xk", [D, L], BF16, isOutput=False)
    xv = nc.declare_dram_parameter("xv", [D, L], BF16, isOutput=False)
    wq = nc.declare_dram_parameter("wq", [P, NDT, DS], BF16, isOutput=False)
    wk = nc.declare_dram_parameter("wk", [P, NDT, DS], BF16, isOutput=False)
    wv = nc.declare_dram_parameter("wv", [P, NDT, DS], BF16, isOutput=False)
    wp = nc.declare_dram_parameter("wp", [P, 2, D], BF16, isOutput=False)
    bqp = nc.declare_dram_parameter("bq", [P, 2], F32, isOutput=False)
    bkp = nc.declare_dram_parameter("bk", [P, 2], F32, isOutput=False)
    emask = nc.declare_dram_parameter("emask", [P, LT], BF16, isOutput=False)
    cmask = nc.declare_dram_parameter("cmask", [P, 4, 512], BF16, isOutput=False)
    out = nc.declare_dram_parameter("out", [L, D], BF16, isOutput=True)

    with tile.TileContext(nc) as tc:
        with tc.tile_pool(name="consts", bufs=1) as consts, \
             tc.tile_pool(name="state", bufs=1) as state, \
             tc.tile_pool(name="xp", bufs=10) as xp, \
             tc.tile_pool(name="ptp", bufs=8) as ptp, \
             tc.tile_pool(name="rp", bufs=2) as rp, \
             tc.tile_pool(name="op", bufs=2) as op, \
             tc.tile_pool(name="pw", bufs=3, space="PSUM") as pw, \
             tc.tile_pool(name="py", bufs=2, space="PSUM") as py:

            # ---- constants ----
            wq_sb = consts.tile([P, NDT, DS], BF16)
            wk_sb = consts.tile([P, NDT, DS], BF16)
            wv_sb = consts.tile([P, NDT, DS], BF16)
            for (wsb_, wdr_) in ((wv_sb, wv), (wq_sb, wq), (wk_sb, wk)):
                for dt2 in range(0, NDT, 2):
                    nc.scalar.dma_start(wsb_[:, dt2:dt2 + 2, :], wdr_[:, dt2:dt2 + 2, :])
            wp_sb = consts.tile([P, 2, D], BF16)
            nc.scalar.dma_start(wp_sb[:], wp[:])
            bq_sb = consts.tile([P, 2], F32)
            bk_sb = consts.tile([P, 2], F32)
            nc.scalar.dma_start(bq_sb[:], bqp[:])
            nc.scalar.dma_start(bk_sb[:], bkp[:])
            em_sb = consts.tile([P, LT], BF16)
            nc.scalar.dma_start(em_sb[:], emask[:])
            cm_sb = consts.tile([P, 4, 512], BF16)
            nc.scalar.dma_start(cm_sb[:], cmask[:])

            # ---- PE warm-up during input DMA lead-in (results never read) ----
            wu = consts.tile([P, 512], BF16)
            nc.any.memset(wu[:], 0.25)
            for i in range(32):
                pwu = pw.tile([P, 512], F32, tag="pw")
                nc.tensor.matmul(pwu[:], wu[:, 0:128], wu[:], start=True, stop=True)

            # ---- big state ----
            QT = state.tile([P, 2, L], BF16)       # [dout pair, chunk, l]
            KT = state.tile([P, 2, L], BF16)
            V = state.tile([P, LT, HPC * (HD + 1)], BF16)  # per head: [V(64) | emask]
            YT = state.tile([P, 2, L], BF16)       # normalized y^T, head pairs

            # "ones" (key-valid mask) columns of V
            for h in range(HPC):
                col = 65 * h + HD
                nc.scalar.copy(V[:, :, col:col + 1], em_sb[:, :, None])

            # ---- V projection: V[l, dout] natural layout ----
            for j in range(NJ):
                xt = []
                for dt in range(NDT):
                    t = xp.tile([P, 512], BF16, tag="x")
                    nc.sync.dma_start(t[:], xv[ds(P * dt, P), ds(512 * j, 512)])
                    xt.append(t)
                for sub in range(4):
                    lt = 4 * j + sub
                    pv = pw.tile([P, DS], F32, tag="pw")
                    for dt in range(NDT):
                        nc.tensor.matmul(pv[:], xt[dt][:, ds(128 * sub, 128)],
                                         wv_sb[:, dt, :], start=(dt == 0),
                                         stop=(dt == NDT - 1))
                    for h in range(HPC):
                        if h % 2 == 0:
                            nc.scalar.copy(V[:, lt, ds(65 * h, HD)],
                                           pv[:, ds(HD * h, HD)])
                        else:
                            nc.vector.tensor_copy(V[:, lt, ds(65 * h, HD)],
                                                  pv[:, ds(HD * h, HD)])

            # ---- Q/K projections, interleaved per j-chunk ----
            for j in range(NJ):
                for (xin, wsb, bsb, dst) in ((xq, wq_sb, bq_sb, QT),
                                             (xk, wk_sb, bk_sb, KT)):
                    xt = []
                    for dt in range(NDT):
                        t = xp.tile([P, 512], BF16, tag="x")
                        nc.sync.dma_start(t[:], xin[ds(P * dt, P), ds(512 * j, 512)])
                        xt.append(t)
                    for dc in range(2):
                        pacc = pw.tile([P, 512], F32, tag="pw")
                        for dt in range(NDT):
                            nc.tensor.matmul(pacc[:], wsb[:, dt, ds(128 * dc, 128)],
                                             xt[dt][:], start=(dt == 0),
                                             stop=(dt == NDT - 1))
                        nc.scalar.activation(dst[:, dc, ds(512 * j, 512)], pacc[:],
                                             mybir.ActivationFunctionType.Identity,
                                             bias=bsb[:, dc:dc + 1])

            # ---- attention per head ----
            for h in range(HPC):
                hc, hrow = h // 2, 64 * (h % 2)
                for j in range(NJ):
                    nk = 4 * j + 4
                    pyt = py.tile([65, 512], F32, tag="py")
                    for blk in range(nk // 2):
                        t0 = 2 * blk
                        # last diagonal pair: q-cols [0:256) of this chunk are
                        # fully causal-masked -> skip computing them
                        trim = 256 if t0 - 4 * j == 2 else 0
                        qw = 512 - trim
                        pwt = pw.tile([P, 1024], F32, tag="pw")
                        for u in range(2):
                            nc.tensor.matmul(pwt[:, ds(512 * u, qw)],
                                             KT[ds(hrow, HD), hc, ts(t0 + u, P)],
                                             QT[ds(hrow, HD), hc,
                                                ds(512 * j + trim, qw)],
                                             start=True, stop=True)
                        pt = ptp.tile([P, 1024], BF16, tag="pt")
                        pwv = pwt[:].rearrange("p (a b) -> p a b", a=2)[:, :, 0:qw]
                        ptv = pt[:, ds(0, 2 * qw)].rearrange("p (a b) -> p a b", a=2)
                        nc.scalar.activation(ptv, pwv,
                                             mybir.ActivationFunctionType.Exp)
                        if t0 >= 4 * j:
                            r = t0 - 4 * j
                            nc.vector.tensor_mul(out=ptv, in0=ptv,
                                                 in1=cm_sb[:, r:r + 2, trim:512])
                        for u in range(2):
                            t = t0 + u
                            nc.tensor.matmul(pyt[:, ds(trim, qw)],
                                             V[:, t, ds(65 * h, HD + 1)],
                                             pt[:, ds(u * qw, qw)],
                                             start=(t == 0), stop=(t == nk - 1))
                    # normalization: R = approx(1/sums) broadcast over partitions
                    rrow = rp.tile([P, 512], F32, tag="rr")
                    nc.vector.tensor_copy(rrow[0:1, :], pyt[64:65, :])
                    Rb = rp.tile([P, 512], F32, tag="rb")
                    nc.gpsimd.partition_broadcast(Rb[:], rrow[0:1, :])
                    Rr = rp.tile([P, 512], F32, tag="rc")
                    nc.vector.reciprocal_approx_fast(Rr[:], Rb[:])
                    nc.vector.tensor_mul(out=YT[ds(hrow, HD), hc, ds(512 * j, 512)],
                                         in0=pyt[0:64, :],
                                         in1=Rr[ds(hrow, HD), :])

            # ---- output projection ----
            for lt in range(LT):
                osb = op.tile([P, D], BF16, tag="o")
                for dc in range(2):
                    po = pw.tile([P, 512], F32, tag="pw")
                    for hc in range(2):
                        nc.tensor.matmul(po[:], YT[:, hc, ts(lt, P)],
                                         wp_sb[:, hc, ds(512 * dc, 512)],
                                         start=(hc == 0), stop=(hc == 1))
                    if dc == 0:
                        nc.scalar.copy(osb[:, ds(512 * dc, 512)], po[:])
                    else:
                        nc.vector.tensor_copy(osb[:, ds(512 * dc, 512)], po[:])
                nc.sync.dma_start(out[ts(lt, P), :], osb[:])

    nc.compile()
    return nc


def _get_nc():
    if "nc" not in _CACHE:
        _CACHE["nc"] = _build()
    return _CACHE["nc"]


def _wlayout(w):
    # [D, DS] -> [P, NDT, DS] with row (o*P + p) at [p, o, :]
    return np.ascontiguousarray(
        w.reshape(NDT, P, DS).transpose(1, 0, 2).astype(ml_dtypes.bfloat16))


def _wlayout2(w):
    # [DS, D] -> [P, 2, D]
    return np.ascontiguousarray(
        w.reshape(2, P, D).transpose(1, 0, 2).astype(ml_dtypes.bfloat16))


def _shard_inputs(query, key, value, kmask, Wq, bq, Wk, bk, Wv, Wp):
    kk = np.arange(P)[:, None, None]
    rr = np.arange(4)[None, :, None]
    qq = np.arange(512)[None, None, :]
    cmask = (P * rr + kk <= qq).astype(ml_dtypes.bfloat16)
    scale = 0.125  # 1/sqrt(HD), folded into the Q projection
    bf = ml_dtypes.bfloat16
    in_maps = []
    for c in range(NCORES):
        b, hg = divmod(c, HPC)
        hs = slice(DS * hg, DS * (hg + 1))
        kvalid = kmask[b].astype(np.float32)
        in_maps.append({
            "xq": np.ascontiguousarray(query[b].T.astype(bf)),
            "xk": np.ascontiguousarray(key[b].T.astype(bf)),
            "xv": np.ascontiguousarray((value[b] * kvalid[:, None]).T.astype(bf)),
            "wq": _wlayout(Wq[hs].T * scale),
            "wk": _wlayout(Wk[hs].T),
            "wv": _wlayout(Wv[hs].T),
            "wp": _wlayout2(Wp[:, hs].T),
            "bq": np.ascontiguousarray((bq[hs] * scale).reshape(2, P).T),
            "bk": np.ascontiguousarray(bk[hs].reshape(2, P).T),
            "emask": np.ascontiguousarray(kvalid.reshape(LT, P).T.astype(bf)),
            "cmask": cmask,
        })
    return in_maps


def kernel(query, key, value, kmask, Wq, bq, Wk, bk, Wv, bv, Wp, bp):
    query = np.asarray(query, dtype=np.float32)
    key = np.asarray(key, dtype=np.float32)
    value = np.asarray(value, dtype=np.float32)
    kmask = np.asarray(kmask)
    Wq = np.asarray(Wq, dtype=np.float32)
    bq = np.asarray(bq, dtype=np.float32)
    Wk = np.asarray(Wk, dtype=np.float32)
    bk = np.asarray(bk, dtype=np.float32)
    Wv = np.asarray(Wv, dtype=np.float32)
    bv = np.asarray(bv, dtype=np.float32)
    Wp = np.asarray(Wp, dtype=np.float32)
    bp = np.asarray(bp, dtype=np.float32)

    in_maps = _shard_inputs(query, key, value, kmask, Wq, bq, Wk, bk, Wv, Wp)
    nc = _get_nc()
    res = run_bass_kernel_spmd(nc, in_maps, list(range(NCORES))).results

    outp = np.zeros((B, L, D), dtype=np.float32)
    for c in range(NCORES):
        b = c // HPC
        outp[b] += res[c]["out"].astype(np.float32)
    outp += bp[None, None, :] + (bv @ Wp.T)[None, None, :]
    return outp
